# revision 43
# baseline (speedup 1.0000x reference)
"""Multi-head attention (S=2048, B=2, D=1024, H=16, Hd=64) on 8 trn2 cores.

Sharding: core = (batch b, head-group g of 4 heads)  -> 2*4 = 8 cores.
Each core computes the full attention for its 4 heads / 1 batch and a
partial output projection (row-parallel Wo); the host sums the 4 partials
per batch and adds bo (+ the host-folded bv @ Wo term).

v3 design (from v2 trace analysis: kernel was ACT/DVE-elementwise-bound,
which starved the PE and kept the HAM clock gate at half rate):
  - attention runs in 8 blocks of 512 queries; per t-tile each head gets
    ONE [128,512] score mm (pair-packed via tile_position), ONE exp op
    (head0 -> ACT exact exp, head1 -> DVE Schraudolph), ONE av mm.
    Per-t engine budgets: PE ~740ns, ACT ~720ns, DVE ~690ns - balanced.
  - dedicated PSUM tags: score 2x[128,512] + chain 2x[128,512] +
    out/proj 2x[128,1024] = exactly 8 banks; out-projection no longer
    competes with the score pipeline for PSUM.
  - out-projection in fp16 (attn2, Wo, partials): v2's f32r ran as
    fp32_mode=HIGH matmuls (~4x slower, FWL-poisoning). Output partials
    DMA'd as fp16 (half the tail traffic); host accumulates in f32.
  - v-bias folded host-side: attn@Wo partials exclude bv; the host adds
    bo + bv@Wo once (exact identity).
  - proj drains / out-proj copies are [1024]-wide single ops.
  - x DMAs split across sync/vector/scalar engine queues so the three
    input tensors transfer in parallel.
"""

import math
import sys

for _p in ("/opt/trn_rl_repo", "/root/.axon_site/_ro/trn_rl_repo"):
    if _p not in sys.path:
        sys.path.insert(0, _p)

import numpy as np
import ml_dtypes

S = 2048
B = 2
D = 1024
H = 16
HD = 64
NH = 4  # heads per core
P = 128
KD = D // P  # 8 contraction tiles for projections

BF16 = ml_dtypes.bfloat16
F16 = np.float16

# Schraudolph exp-as-int-bits constants (fp16 bit pattern via int16):
#   bits = round(score * (1024/ln2)/8 + (15*1024 + C))
SCHRAUDOLPH_A = 1024.0 / math.log(2.0) / 8.0
SCHRAUDOLPH_C = -44.0

_BUILD_CACHE = {}


def build_bass(s=S, debug_taps=False, no_ldw=True):
    """Build the per-core Bass module (same program for all 8 cores)."""
    import concourse.bacc as bacc
    import concourse.bass as bass
    import concourse.mybir as mybir
    import concourse.tile as tile

    f32 = mybir.dt.float32
    f16 = mybir.dt.float16
    bf16 = mybir.dt.bfloat16
    i16 = mybir.dt.int16
    AF = mybir.ActivationFunctionType
    ALU = mybir.AluOpType

    NT = s // P            # 16 key tiles
    QS = 512               # queries per attention block
    NQ = s // QS           # 4 query blocks per head pair
    PW = 1024              # projection drain width
    NPW = s // PW          # 2 proj rounds per p
    KH = KD // 2

    nc = bacc.Bacc("TRN2", target_bir_lowering=False, debug=False, num_devices=8)

    xq = nc.dram_tensor("xq_t", [D, s], f16, kind="ExternalInput").ap()
    xk = nc.dram_tensor("xk_t", [D, s], f16, kind="ExternalInput").ap()
    xv = nc.dram_tensor("xv_t", [D, s], f16, kind="ExternalInput").ap()
    wq = nc.dram_tensor("wq_t", [D, 256], f16, kind="ExternalInput").ap()
    wk = nc.dram_tensor("wk_t", [D, 256], f16, kind="ExternalInput").ap()
    wv = nc.dram_tensor("wv_t", [D, 256], f16, kind="ExternalInput").ap()
    wo = nc.dram_tensor("wo_h", [P, 2, D], f16, kind="ExternalInput").ap()
    bq2 = nc.dram_tensor("bq2", [P, 2], f32, kind="ExternalInput").ap()
    bk2 = nc.dram_tensor("bk2", [P, 2], f32, kind="ExternalInput").ap()
    out = nc.dram_tensor("out", [s, D], f16, kind="ExternalOutput").ap()

    from contextlib import ExitStack

    no_ldw_insts = []

    def _mark_no_ldw(mm):
        if not no_ldw:
            return
        inst = getattr(mm, "ins", mm)
        inst.ldweights = False
        no_ldw_insts.append(inst.name)

    with tile.TileContext(nc) as tc, ExitStack() as ctx:
        consts = ctx.enter_context(tc.tile_pool(name="consts", bufs=1))
        persist = ctx.enter_context(tc.tile_pool(name="persist", bufs=1))
        xpool = ctx.enter_context(tc.tile_pool(name="xpool", bufs=3))
        epool = ctx.enter_context(tc.tile_pool(name="epool", bufs=8))
        rzpool = ctx.enter_context(tc.tile_pool(name="rzpool", bufs=2))
        ospool = ctx.enter_context(tc.tile_pool(name="ospool", bufs=3))
        drampool = ctx.enter_context(tc.tile_pool(name="drampool", bufs=2, space="DRAM"))
        # PSUM: score 4x1 bank + chain 2x1 bank + out/proj 1x2 banks = 8
        psp = ctx.enter_context(tc.tile_pool(name="psp", bufs=2, space="PSUM"))
        accp = ctx.enter_context(tc.tile_pool(name="accp", bufs=2, space="PSUM"))

        # ---- PE warm-up + ACT exp-table priming (runs during x DMA) ----
        warm = consts.tile([P, QS], f16, name="warm")
        nc.vector.memset(warm, 0.0)
        prime = consts.tile([1, 8], f32, name="prime")
        nc.scalar.activation(prime, warm[0:1, 0:8], AF.Exp, bias=0.0, scale=1.0)
        wps = psp.tile([P, QS], f32, tag="score", name="warm_ps", bufs=4)
        for _ in range(18):
            nc.tensor.matmul(wps, lhsT=warm[:, 0:P], rhs=warm, start=True, stop=True)

        # ---- constants + x loads, split across DMA queues ---------------
        # x tensors load in s-halves (full k range): each half feeds a
        # whole projection round the moment it lands. Queue order is
        # deadline-driven; aggregate HBM bandwidth is the real limit.
        S2 = s // 2

        def load_x_shalf(xdram, name, h, eng):
            xt = xpool.tile([P, KD, S2], f16, tag="x", name=f"{name}{h}", bufs=6)
            eng.dma_start(
                out=xt,
                in_=xdram.rearrange("(k p) s -> p k s", p=P)[:, :, h * S2:(h + 1) * S2],
            )
            return xt

        # sync queue (measured ~3x faster than the scalar/gpsimd queues):
        # everything deadline-critical, in deadline order.
        wk_sb = consts.tile([P, KD, 256], f16, name="wk_sb")
        nc.sync.dma_start(out=wk_sb, in_=wk.rearrange("(k p) e -> p k e", p=P))
        bq_sb = consts.tile([P, 2], f32, name="bq_sb")
        nc.sync.dma_start(out=bq_sb, in_=bq2)
        bk_sb = consts.tile([P, 2], f32, name="bk_sb")
        nc.sync.dma_start(out=bk_sb, in_=bk2)
        xk_sb = [load_x_shalf(xk, "xk_sb", h, nc.sync) for h in range(2)]
        xv_sb = [None, None]
        xq_sb = [None, None]
        wv_sb = consts.tile([P, KD, 256], f16, name="wv_sb")
        nc.sync.dma_start(out=wv_sb, in_=wv.rearrange("(k p) e -> p k e", p=P))
        xv_sb[0] = load_x_shalf(xv, "xv_sb", 0, nc.sync)
        xv_sb[1] = load_x_shalf(xv, "xv_sb", 1, nc.sync)

        # scalar queue (~2.5x slower): the whole q path
        wq_sb = consts.tile([P, KD, 256], f16, name="wq_sb")
        nc.scalar.dma_start(out=wq_sb, in_=wq.rearrange("(k p) e -> p k e", p=P))
        xq_sb[0] = load_x_shalf(xq, "xq_sb", 0, nc.scalar)
        xq_sb[1] = load_x_shalf(xq, "xq_sb", 1, nc.scalar)

        # gpsimd queue: wo only (needed ~halfway in)
        wo_sb = consts.tile([P, 2, D], f16, name="wo_sb")
        nc.gpsimd.dma_start(out=wo_sb, in_=wo)

        # ---- persistent activations -----------------------------------
        q2 = persist.tile([P, 2, s], f16, name="q2")
        k2 = persist.tile([P, 2, s], f16, name="k2")
        v_aug = persist.tile([P, NH, NT, 65], f16, name="v_aug")
        nc.vector.memset(v_aug, 1.0)  # col 64 stays 1.0 = Z ones column
        # attn2: pair-packed normalized attention [128(e of 2 heads), 2, s]
        attn2 = persist.tile([P, 2, s], f16, name="attn2")

        def proj_round(xh, w_sb, b_sb, dst, p, w):
            # dst[:, p, w-slice] = ((x @ W_pair.T)^T + bias), PW-wide drain.
            # w selects the s-half; xh[w] holds exactly that half (all k).
            pss = psp.tile([P, PW], f32, tag="out", name="qkps", bufs=1)
            for k in range(KD):
                for c in range(2):
                    mm = nc.tensor.matmul(
                        pss[:, c * QS:(c + 1) * QS],
                        lhsT=w_sb[:, k, p * P:(p + 1) * P],
                        rhs=xh[w][:, k, c * QS:(c + 1) * QS],
                        start=(k == 0),
                        stop=(k == KD - 1),
                    )
                    if c > 0:
                        _mark_no_ldw(mm)
            # bias add on ACT (Identity + per-partition bias AP), 1024-wide
            nc.scalar.add(dst[:, p, w * PW:(w + 1) * PW], pss, b_sb[:, p:p + 1])

        def v_round(xh, t):
            ps = psp.tile([P, 256], f32, tag="score", name="vps", bufs=4)
            h, tt = t // (NT // 2), t % (NT // 2)
            for k in range(KD):
                nc.tensor.matmul(
                    ps,
                    lhsT=xh[h][:, k, tt * P:(tt + 1) * P],
                    rhs=wv_sb[:, k, :],
                    start=(k == 0),
                    stop=(k == KD - 1),
                )
            # v bias folded host-side: plain strided drain on DVE
            nc.vector.tensor_copy(
                v_aug[:, :, t, 0:64],
                ps.rearrange("p (h e) -> p h e", h=NH),
            )

        def out_proj(sc_i, copy_dve=False, chain_slots=False):
            # chain_slots: after the last attention block the chain PSUM
            # banks are free; use two of them so end tiles double-buffer
            # against the single "out" slot.
            if chain_slots:
                ops = [
                    accp.tile([P, QS], f32, tag="chain", name=f"opc{i}")
                    for i in range(2)
                ]
            else:
                op = psp.tile([P, PW], f32, tag="out", name="op", bufs=1)
                ops = [op[:, 0:QS], op[:, QS:PW]]
            # p outer: both nh chunks share the attn2 stationary per p
            for pp in range(2):
                for nh_i in range(2):
                    mm = nc.tensor.matmul(
                        ops[nh_i],
                        lhsT=attn2[:, pp, sc_i * P:(sc_i + 1) * P],
                        rhs=wo_sb[:, pp, nh_i * QS:(nh_i + 1) * QS],
                        start=(pp == 0),
                        stop=(pp == 1),
                    )
                    if nh_i > 0:
                        _mark_no_ldw(mm)
            ob = ospool.tile([P, D], f16, tag="ob", name="ob")
            if chain_slots:
                # split the drain across both elementwise engines
                nc.scalar.copy(ob[:, 0:QS], ops[0])
                nc.vector.tensor_copy(ob[:, QS:PW], ops[1])
            elif copy_dve:
                nc.vector.tensor_copy(ob, op)
            else:
                nc.scalar.copy(ob, op)
            # alternate output DMA queues so tail tiles don't serialize
            oeng = nc.sync if sc_i % 2 == 0 else nc.scalar
            oeng.dma_start(out=out[sc_i * P:(sc_i + 1) * P, :], in_=ob)

        def normalize(p, soff, chains, last=False):
            # attn = attn~ / Z ; Z sits in row 64 of each chain tile.
            # Drain the WHOLE chain to SBUF on ACT (partition-parallel) to
            # release the chain PSUM banks early, then Z broadcast via DRAM
            # bounce, recip+mult on DVE from SBUF. For the last block no
            # one needs the banks - normalize straight from PSUM instead.
            rzs, csbs, zrows = [], [], []
            for hi in range(2):
                if last:
                    # only the Z row moves to SBUF (DMA can't read PSUM);
                    # the mults below read the chain PSUM directly
                    csb = chains[hi]
                    zr = rzpool.tile([1, QS], f32, tag="zr", name=f"zr{hi}")
                    nc.scalar.copy(zr, chains[hi][64:65, :])
                else:
                    csb = rzpool.tile([P, QS], f32, tag="chn", name=f"chn{hi}")
                    # split the chain drains across both PSUM-capable
                    # engines so neither stacks two 720ns ops ahead of the
                    # next block's first exp
                    if hi == 0:
                        nc.scalar.copy(csb[0:65, :], chains[hi][0:65, :])
                    else:
                        nc.vector.tensor_copy(csb[0:65, :], chains[hi][0:65, :])
                    zr = csb[64:65, :]
                csbs.append(csb)
                zrows.append(zr)
                rzs.append(rzpool.tile([P, QS], f32, tag="rz", name=f"rz{hi}"))
            for hi in range(2):
                if last:
                    # GpSimd partition broadcast: no DRAM round-trip on the
                    # tail-critical path
                    nc.gpsimd.partition_broadcast(rzs[hi][0:64, :], zrows[hi])
                else:
                    zd = drampool.tile([1, QS], f32, tag="zd", name=f"zd{hi}")
                    nc.sync.dma_start(out=zd, in_=zrows[hi])
                    zbc = bass.AP(
                        tensor=zd.tensor,
                        offset=zd.offset,
                        ap=[[0, 64]] + list(zd.ap[-1:]),
                    )
                    nc.sync.dma_start(out=rzs[hi][0:64, :], in_=zbc)
            atmp = rzpool.tile([HD, QS], f16, tag="atmp", name="atmp", bufs=1)
            for hi in range(2):
                # recip at base partition 0 (base 64 miscomputes on HW)
                nc.vector.reciprocal_approx_fast(rzs[hi][0:64, :], rzs[hi][0:64, :])
            # even head of pair -> attn2 rows 0:64 directly
            nc.vector.tensor_tensor(
                attn2[0:64, p, soff:soff + QS],
                csbs[0][0:64, :],
                rzs[0][0:64, :],
                ALU.mult,
            )
            # odd head: drain to tmp then DMA-shift to rows 64:128; chunked
            # on the last block so the first end out-projs start earlier
            nch = 2 if last else 1
            cw = QS // nch
            for c in range(nch):
                cs = slice(c * cw, (c + 1) * cw)
                nc.vector.tensor_tensor(
                    atmp[:, cs], csbs[1][0:64, cs], rzs[1][0:64, cs], ALU.mult
                )
                nc.sync.dma_start(
                    out=attn2[64:128, p, soff + c * cw:soff + (c + 1) * cw],
                    in_=atmp[:, cs],
                )

        def attn_block(p, qs, filler_map=None, end_filler=(), v_map=None,
                       last=False):
            soff = qs * QS
            heads = (2 * p, 2 * p + 1)
            chains = [
                accp.tile([P, QS], f32, tag="chain", name=f"ch{hi}")
                for hi in range(2)
            ]

            def issue_scores(t):
                sc = {}
                for hi in range(2):
                    rlo = 64 * hi
                    ps_t = psp.tile([P, QS], f32, tag="score", name=f"sc{hi}", bufs=4)
                    nc.tensor.matmul(
                        ps_t,
                        lhsT=k2[rlo:rlo + 64, p, t * P:(t + 1) * P],
                        rhs=q2[rlo:rlo + 64, p, soff:soff + QS],
                        start=True,
                        stop=True,
                        tile_position=(rlo, 0),
                    )
                    sc[hi] = ps_t
                return sc

            # software pipeline: scores run one t ahead so the PE never
            # sits behind an exp-semaphore wait in its in-order queue
            sc = issue_scores(0)
            for t in range(NT):
                nxt = issue_scores(t + 1) if t + 1 < NT else None
                et = {}
                for hi in range(2):
                    e = epool.tile([P, QS], f16, tag="exp", name=f"e{hi}")
                    # 7/16 of tiles take the approximate DVE exp; ACT has
                    # slack for 9/16 exact (PE is the block bottleneck)
                    if (t + hi) % 2 == 1 and not (t % 8 == 0 and hi == 1):
                        # Schraudolph approximate exp on the Vector engine
                        nc.vector.tensor_scalar(
                            e.bitcast(i16),
                            sc[hi],
                            SCHRAUDOLPH_A,
                            15.0 * 1024.0 + SCHRAUDOLPH_C,
                            ALU.mult,
                            ALU.add,
                        )
                    else:
                        nc.scalar.activation(
                            e, sc[hi], AF.Exp, bias=0.0, scale=0.125
                        )
                    et[hi] = e
                for hi in range(2):
                    nc.tensor.matmul(
                        chains[hi][0:65, :],
                        lhsT=v_aug[:, heads[hi], t, :],
                        rhs=et[hi],
                        start=(t == 0),
                        stop=(t == NT - 1),
                    )
                # ready out-proj tiles / late v rounds ride in the gaps
                if filler_map and t in filler_map:
                    out_proj(filler_map[t], copy_dve=(filler_map[t] % 2 == 1))
                if v_map and t in v_map:
                    v_round(xv_sb, v_map[t])
                sc = nxt
            normalize(p, soff, chains, last=last)
            for sc_i in end_filler:
                out_proj(sc_i, copy_dve=(sc_i % 2 == 1))

        # ---- program order --------------------------------------------
        # deadline-ordered: p0 projections first so block (p0,0) starts as
        # soon as xk + xv half 0 + xq half 0 have landed; v rounds 8-15 and
        # the p1 / second-half projections hide inside the running blocks.
        proj_round(xk_sb, wk_sb, bk_sb, k2, 0, 0)
        proj_round(xk_sb, wk_sb, bk_sb, k2, 0, 1)
        proj_round(xk_sb, wk_sb, bk_sb, k2, 1, 0)
        proj_round(xk_sb, wk_sb, bk_sb, k2, 1, 1)
        for t in range(NT // 2):
            v_round(xv_sb, t)
        proj_round(xq_sb, wq_sb, bq_sb, q2, 0, 0)
        attn_block(0, 0, v_map={5 + j: 8 + j for j in range(NT // 2)})
        attn_block(0, 1)
        proj_round(xq_sb, wq_sb, bq_sb, q2, 1, 0)
        attn_block(1, 0)
        attn_block(1, 1)
        proj_round(xq_sb, wq_sb, bq_sb, q2, 0, 1)
        attn_block(0, 2, filler_map={3: 0, 9: 1})
        attn_block(0, 3, filler_map={3: 2, 9: 3})
        proj_round(xq_sb, wq_sb, bq_sb, q2, 1, 1)
        attn_block(1, 2, filler_map={3: 4, 9: 5})
        attn_block(1, 3, filler_map={2: 6, 5: 7, 8: 8, 11: 9, 14: 10},
                   last=True)
        for j, sc_i in enumerate((11, 12, 13, 14, 15)):
            out_proj(sc_i, copy_dve=(sc_i % 2 == 1), chain_slots=(j % 2 == 1))

        if debug_taps:
            dq2 = nc.dram_tensor("dbg_q2", [P, 2, s], f16, kind="ExternalOutput").ap()
            nc.sync.dma_start(out=dq2, in_=q2)
            dk2 = nc.dram_tensor("dbg_k2", [P, 2, s], f16, kind="ExternalOutput").ap()
            nc.sync.dma_start(out=dk2, in_=k2)
            dva = nc.dram_tensor("dbg_vaug", [P, NH, NT, 65], f16, kind="ExternalOutput").ap()
            nc.sync.dma_start(out=dva, in_=v_aug)
            dat = nc.dram_tensor("dbg_attn", [P, 2, s], f16, kind="ExternalOutput").ap()
            nc.sync.dma_start(out=dat, in_=attn2)

    nc.compile()

    # Safety: every no-LDWEIGHTS matmul must immediately follow (in PE
    # program order) a matmul/ldweights with the same stationary operand.
    if no_ldw_insts:
        flagged = set(no_ldw_insts)

        def weights_key(inst):
            op = inst.opcode
            if op == "Matmult":
                w = inst.ins[1]
            elif op == "Ldweights":
                w = inst.ins[0]
            else:
                return None
            return repr(w)

        bad = []
        import concourse.mybir as mybir_
        for fn in nc.m.functions:
            for blk_ in fn.blocks:
                prev_w = None
                for inst in blk_.instructions:
                    if getattr(inst, "engine", None) != mybir_.EngineType.PE:
                        continue
                    wk_ = weights_key(inst)
                    if wk_ is None:
                        continue
                    if inst.name in flagged and wk_ != prev_w:
                        bad.append(inst.name)
                    prev_w = wk_
        if bad:
            raise RuntimeError(
                f"no-ldweights matmuls not adjacent to their weight load: "
                f"{bad[:5]} ({len(bad)} total)"
            )
    return nc


def get_bass(s=S):
    if s not in _BUILD_CACHE:
        try:
            _BUILD_CACHE[s] = build_bass(s)
        except RuntimeError:
            # scheduler broke a no-ldweights adjacency: rebuild without
            # the weight-load elision (correct, slightly slower)
            _BUILD_CACHE[s] = build_bass(s, no_ldw=False)
    return _BUILD_CACHE[s]


def make_in_maps(query, key, value, Wq, bq, Wk, bk, Wv, bv, Wo):
    """Host-side sharding: per-core input dict for core = b*4 + g."""
    in_maps = []
    for core in range(8):
        b, g = core // 4, core % 4
        cs = slice(g * 256, (g + 1) * 256)
        # pair-packed: wo_h[hd + 64*(h%2), h//2, :] = Wo[:, g*256 + h*64 + hd]
        wo_h = (
            np.ascontiguousarray(Wo[:, cs].T)  # [256(h*64+hd), 1024]
            .reshape(2, P, D)
            .transpose(1, 0, 2)
        )
        m = {
            "xq_t": np.ascontiguousarray(query[:, b, :].T).astype(F16),
            "xk_t": np.ascontiguousarray(key[:, b, :].T).astype(F16),
            "xv_t": np.ascontiguousarray(value[:, b, :].T).astype(F16),
            "wq_t": np.ascontiguousarray(Wq[cs, :].T).astype(F16),
            "wk_t": np.ascontiguousarray(Wk[cs, :].T).astype(F16),
            "wv_t": np.ascontiguousarray(Wv[cs, :].T).astype(F16),
            "wo_h": np.ascontiguousarray(wo_h).astype(F16),
            "bq2": np.ascontiguousarray(bq[cs].reshape(2, P).T).astype(np.float32),
            "bk2": np.ascontiguousarray(bk[cs].reshape(2, P).T).astype(np.float32),
        }
        in_maps.append(m)
    return in_maps


def kernel(query, key, value, Wq, bq, Wk, bk, Wv, bv, Wo, bo):
    from concourse.bass_utils import run_bass_kernel_spmd

    query = np.asarray(query, dtype=np.float32)
    key = np.asarray(key, dtype=np.float32)
    value = np.asarray(value, dtype=np.float32)
    Wq = np.asarray(Wq, dtype=np.float32)
    Wk = np.asarray(Wk, dtype=np.float32)
    Wv = np.asarray(Wv, dtype=np.float32)
    Wo = np.asarray(Wo, dtype=np.float32)
    bv = np.asarray(bv, dtype=np.float32)
    bo32 = np.asarray(bo, dtype=np.float32)

    nc = get_bass(S)
    in_maps = make_in_maps(query, key, value, Wq, bq, Wk, bk, Wv, bv, Wo)
    res = run_bass_kernel_spmd(nc, in_maps, core_ids=list(range(8)))
    outs = [res.results[c]["out"] for c in range(8)]

    # v-bias folds through the output projection: softmax(QK)(V + bv) Wo^T
    # = softmax(QK) V Wo^T + bv Wo^T   (softmax rows sum to 1)
    bias_full = bo32 + bv @ Wo.T

    full = np.empty((S, B, D), dtype=np.float32)
    for b in range(B):
        acc = outs[b * 4].astype(np.float32)
        for g in range(1, 4):
            acc += outs[b * 4 + g].astype(np.float32)
        full[:, b, :] = acc + bias_full[None, :]
    return full
